# revision 4
# baseline (speedup 1.0000x reference)
"""Trainium2 kernel for nn_ClusteringLayer (vq_codebook).

Problem: x (1, 131072, 256) f32, cluster_centers (1024, 256) f32.
For each cluster k: find argmin_n ||x[n] - c[k]||^2 and return that x row.
Output: (1, 1024, 256) f32.

Strategy (8 NeuronCores, x sharded along n, centers replicated):
  argmin_n d2[n,k] == argmax_n s[n,k],  s = 2*x.c - |x|^2  (c2[k] const per k)
  Host pre-sorts points by |x|^2, so |x|^2 is nearly constant inside each
  contiguous 2048-point group. The device then needs no x2 at all:
    psum[k_tile, grp] = bf16 matmul  xT_sorted (moving) x (2C)T (stationary)
    VectorE reduce_max over each group directly from PSUM -> bmax2dot f32.
  Host recovery per cluster:
    upper/lower bounds of the true group max of s from bmax2dot and the
    group's [x2min, x2max]; every group whose upper bound reaches the best
    lower bound - THETA is rescored exactly (fp32 gemm + fp64 refine,
    first-original-index tiebreak). Exactness relies only on bounds +
    THETA covering the bf16 matmul noise (~0.12 abs, validated).
"""

import os
import sys

for _p in ("/opt/trn_rl_repo",):
    if os.path.isdir(_p) and _p not in sys.path:
        sys.path.append(_p)

import numpy as np
import ml_dtypes

import concourse.bass as bass
import concourse.bacc as bacc
import concourse.mybir as mybir
import concourse.tile as tile

NCORES = 8
N = 131072
F = 256
K = 1024
SH = N // NCORES            # 16384 points per core
GRP = 2048                  # group size for the device-side max reduction
NG = SH // GRP              # 8 groups per core
NGRP = NCORES * NG          # 64 groups total
KT = K // 128               # 8 cluster tiles
NCH = F // 128              # 2 contraction chunks
THETA = 1.5                 # host rescue radius (covers bf16 score noise)
TOPM = 32                   # fp32->fp64 refine width per (cluster, group)
DIRECT_KT = 2               # k-tiles per group reduced directly from PSUM

BF16 = ml_dtypes.bfloat16


def build_nc():
    """Build + compile the per-core Bass program (same program on all cores)."""
    nc = bacc.Bacc("TRN2", target_bir_lowering=False, debug=False,
                   num_devices=NCORES)

    xt = nc.dram_tensor("xt", [NCH, 128, SH], mybir.dt.bfloat16,
                        kind="ExternalInput")
    ct2 = nc.dram_tensor("ct2", [NCH, 128, K], mybir.dt.bfloat16,
                         kind="ExternalInput")
    bmax_d = nc.dram_tensor("bmax", [128, KT * NG], mybir.dt.float32,
                            kind="ExternalOutput")

    with tile.TileContext(nc) as tc:
        with (
            tc.tile_pool(name="consts", bufs=1) as cpool,
            tc.tile_pool(name="xtp", bufs=3) as xpool,
            tc.tile_pool(name="psum", bufs=2, space="PSUM") as ppool,
            tc.tile_pool(name="scrap", bufs=3) as spool,
        ):
            ct2_t = []
            for ch in range(NCH):
                t = cpool.tile([128, K], mybir.dt.bfloat16, tag=f"ct{ch}")
                nc.sync.dma_start(t[:], ct2[ch, :, :])
                ct2_t.append(t)
            bmax_t = cpool.tile([128, KT * NG], mybir.dt.float32, tag="bmax")

            for g in range(NG):
                xg = []
                for ch in range(NCH):
                    t = xpool.tile([128, GRP], mybir.dt.bfloat16, tag=f"xt{ch}")
                    nc.sync.dma_start(t[:], xt[ch, :, g * GRP:(g + 1) * GRP])
                    xg.append(t)

                for kt in range(KT):
                    ps = ppool.tile([128, GRP], mybir.dt.float32, tag="ps")
                    for ch in range(NCH):
                        for blk in range(GRP // 512):
                            nc.tensor.matmul(
                                ps[:, blk * 512:(blk + 1) * 512],
                                lhsT=ct2_t[ch][:, kt * 128:(kt + 1) * 128],
                                rhs=xg[ch][:, blk * 512:(blk + 1) * 512],
                                start=(ch == 0),
                                stop=(ch == NCH - 1),
                            )
                    col = kt * NG + g
                    if kt < DIRECT_KT:
                        # path 1: DVE reduces straight from PSUM (1x)
                        nc.vector.tensor_reduce(
                            out=bmax_t[:, col:col + 1],
                            in_=ps[:],
                            axis=mybir.AxisListType.X,
                            op=mybir.AluOpType.max,
                        )
                    else:
                        # path 2: ScalarE evac f16 + DVE 2x pairwise max folds
                        ev = spool.tile([128, GRP], mybir.dt.float16, tag="ev")
                        nc.scalar.copy(ev[:], ps[:])
                        f1 = spool.tile([128, GRP // 2], mybir.dt.float16,
                                        tag="f1")
                        nc.vector.tensor_tensor(
                            out=f1[:], in0=ev[:, 0:GRP // 2],
                            in1=ev[:, GRP // 2:GRP], op=mybir.AluOpType.max)
                        f2 = spool.tile([128, GRP // 4], mybir.dt.float16,
                                        tag="f2")
                        nc.vector.tensor_tensor(
                            out=f2[:], in0=f1[:, 0:GRP // 4],
                            in1=f1[:, GRP // 4:GRP // 2],
                            op=mybir.AluOpType.max)
                        f3 = spool.tile([128, GRP // 8], mybir.dt.float16,
                                        tag="f3")
                        nc.vector.tensor_tensor(
                            out=f3[:], in0=f2[:, 0:GRP // 8],
                            in1=f2[:, GRP // 8:GRP // 4],
                            op=mybir.AluOpType.max)
                        nc.vector.tensor_reduce(
                            out=bmax_t[:, col:col + 1],
                            in_=f3[:],
                            axis=mybir.AxisListType.X,
                            op=mybir.AluOpType.max,
                        )

            nc.sync.dma_start(bmax_d[:, :], bmax_t[:])

    nc.compile()
    return nc


def host_prep(x, cluster_centers):
    """Sort points by |x|^2; build per-core device inputs."""
    x0 = np.ascontiguousarray(x[0], dtype=np.float32)        # (N, F)
    C = np.ascontiguousarray(cluster_centers, dtype=np.float32)
    x2 = np.einsum('nf,nf->n', x0.astype(np.float64),
                   x0.astype(np.float64))
    order = np.argsort(x2, kind="stable").astype(np.int64)
    xs_all = x0[order]                                        # sorted points
    x2s = x2[order]
    ct2_np = np.ascontiguousarray(
        (2.0 * C).T.astype(BF16)).reshape(NCH, 128, K)
    in_maps = []
    for c in range(NCORES):
        xs = xs_all[c * SH:(c + 1) * SH]
        xt_np = np.ascontiguousarray(xs.T.astype(BF16)).reshape(NCH, 128, SH)
        in_maps.append({"xt": xt_np, "ct2": ct2_np})
    return in_maps, x0, C, order, xs_all, x2s


def host_combine(bmax_cores, x0, C, order, xs_all, x2s):
    """Exact argmin recovery from per-group maxima of 2*dot (sorted points)."""
    x64s = xs_all.astype(np.float64)
    C64 = C.astype(np.float64)
    x2s_32 = x2s.astype(np.float32)

    # bmax_cores[c]: [128, KT*NG] -> cluster k = kt*128 + p, col = kt*NG + g
    bm = np.empty((K, NGRP), dtype=np.float32)
    for c in range(NCORES):
        a = np.asarray(bmax_cores[c]).reshape(128, KT, NG)
        bm[:, c * NG:(c + 1) * NG] = a.transpose(1, 0, 2).reshape(K, NG)

    gb = np.arange(NGRP) * GRP
    x2min = x2s[gb].astype(np.float32)            # sorted -> min is first
    x2max = x2s[gb + GRP - 1].astype(np.float32)

    ub = bm - x2min[None, :]                      # >= true group smax
    lb = bm - x2max[None, :]                      # <= true group smax
    win_lb = lb.max(axis=1)
    flags = ub >= (win_lb[:, None] - THETA)       # (K, NGRP)

    pair_clusters = [[] for _ in range(NGRP)]
    ks_idx, ps_idx = np.nonzero(flags)
    for kk, p in zip(ks_idx, ps_idx):
        pair_clusters[p].append(kk)

    best_val = np.full(K, np.inf)
    best_idx = np.zeros(K, dtype=np.int64)        # original indices
    for p, ks in enumerate(pair_clusters):
        if not ks:
            continue
        base = p * GRP
        pts = xs_all[base:base + GRP]
        d32 = x2s_32[base:base + GRP, None] - 2.0 * (pts @ C[ks].T)
        m = min(TOPM, GRP - 1)
        part = np.argpartition(d32, m, axis=0)[:m]
        for j, kk in enumerate(ks):
            srt = base + part[:, j]
            dv = x2s[srt] - 2.0 * (x64s[srt] @ C64[kk])
            ids = order[srt]                      # original indices
            o = np.lexsort((ids, dv))[0]
            if (dv[o] < best_val[kk]) or (dv[o] == best_val[kk]
                                          and ids[o] < best_idx[kk]):
                best_val[kk] = dv[o]
                best_idx[kk] = ids[o]

    return x0[best_idx][None].astype(np.float32)


_NC_CACHE = {}


def kernel(x, cluster_centers):
    from concourse.bass_utils import run_bass_kernel_spmd

    if "nc" not in _NC_CACHE:
        _NC_CACHE["nc"] = build_nc()
    nc = _NC_CACHE["nc"]

    in_maps, x0, C, order, xs_all, x2s = host_prep(x, cluster_centers)
    res = run_bass_kernel_spmd(nc, in_maps, list(range(NCORES)))
    bmax_cores = [res.results[c]["bmax"] for c in range(NCORES)]
    return host_combine(bmax_cores, x0, C, order, xs_all, x2s)


# revision 16
# speedup vs baseline: 6984.6854x; 6984.6854x over previous
"""Trainium2 kernel for nn_ClusteringLayer (vq_codebook).

Problem: x (1, 131072, 256) f32, cluster_centers (1024, 256) f32.
For each cluster k: find argmin_n ||x[n] - c[k]||^2 and return that x row.
Output: (1, 1024, 256) f32.

Strategy (8 NeuronCores, x sharded along n, centers replicated):
  argmin_n d2[n,k] == argmax_n s[n,k],  s = 2*x.c - |x|^2  (c2[k] const per k)
  Host pre-sorts points by |x|^2, so |x|^2 is nearly constant inside each
  contiguous 2048-point group. The device then needs no x2 at all:
    psum[k_tile, grp] = bf16 matmul  xT_sorted (moving) x (2C)T (stationary)
    VectorE reduce_max over each group directly from PSUM -> bmax2dot f32.
  Host recovery per cluster:
    upper/lower bounds of the true group max of s from bmax2dot and the
    group's [x2min, x2max]; every group whose upper bound reaches the best
    lower bound - THETA is rescored exactly (fp32 gemm + fp64 refine,
    first-original-index tiebreak). Exactness relies only on bounds +
    THETA covering the bf16 matmul noise (~0.12 abs, validated).
"""

import os
import sys

for _p in ("/opt/trn_rl_repo",):
    if os.path.isdir(_p) and _p not in sys.path:
        sys.path.append(_p)

import numpy as np
import ml_dtypes

import concourse.bass as bass
import concourse.bacc as bacc
import concourse.mybir as mybir
import concourse.tile as tile

NCORES = 8
N = 131072
F = 256
K = 1024
SH = N // NCORES            # 16384 points per core
GRP = 2048                  # group size for the device-side max reduction
NG = SH // GRP              # 8 groups per core
NGRP = NCORES * NG          # 64 groups total
KT = K // 128               # 8 cluster tiles
NCH = F // 128              # 2 contraction chunks
THETA = 2.5                 # host rescue radius (covers bf16 score noise)
TOPM = 32                   # fp32->fp64 refine width per (cluster, group)

BF16 = ml_dtypes.bfloat16


def build_nc():
    """Build + compile the per-core Bass program (same program on all cores)."""
    nc = bacc.Bacc("TRN2", target_bir_lowering=False, debug=False,
                   num_devices=NCORES)

    xt = nc.dram_tensor("xt", [NCH, 128, SH], mybir.dt.bfloat16,
                        kind="ExternalInput")
    ct2 = nc.dram_tensor("ct2", [NCH, 128, K], mybir.dt.bfloat16,
                         kind="ExternalInput")
    bmax_d = nc.dram_tensor("bmax", [128, KT * NG], mybir.dt.float32,
                            kind="ExternalOutput")

    with tile.TileContext(nc) as tc:
        with (
            tc.tile_pool(name="consts", bufs=1) as cpool,
            tc.tile_pool(name="xtp", bufs=3) as xpool,
            tc.tile_pool(name="psum", bufs=2, space="PSUM") as ppool,
            tc.tile_pool(name="scrap", bufs=3) as spool,
        ):
            warm_w = cpool.tile([128, 128], mybir.dt.bfloat16, tag="warmw")
            warm_x = cpool.tile([128, 512], mybir.dt.bfloat16, tag="warmx")
            nc.gpsimd.memset(warm_w[:], 0.0)
            nc.gpsimd.memset(warm_x[:], 0.0)
            warm_ps = ppool.tile([128, 512], mybir.dt.float32, tag="ps",
                                 name="warmps")
            for _ in range(24):
                nc.tensor.matmul(warm_ps[:], lhsT=warm_w[:], rhs=warm_x[:],
                                 start=True, stop=True)

            ct2_t = []
            for ch in range(NCH):
                t = cpool.tile([128, K], mybir.dt.bfloat16, tag=f"ct{ch}")
                for h in range(2):
                    nc.sync.dma_start(t[:, h * K // 2:(h + 1) * K // 2],
                                      ct2[ch, :, h * K // 2:(h + 1) * K // 2])
                ct2_t.append(t)
            bmax_t = cpool.tile([128, KT * NG], mybir.dt.float32, tag="bmax")

            for g in range(NG):
                # per-512-block x tiles: finer DMA granularity lets the first
                # matmuls start as soon as one 128KB slice lands
                xg = []
                for ch in range(NCH):
                    blks = []
                    for blk in range(GRP // 512):
                        t = xpool.tile([128, 512], mybir.dt.bfloat16,
                                       tag=f"xt{ch}b{blk}")
                        base = g * GRP + blk * 512
                        nc.sync.dma_start(t[:], xt[ch, :, base:base + 512])
                        blks.append(t)
                    xg.append(blks)

                for kt in range(KT):
                    ps = ppool.tile([128, GRP], mybir.dt.float32, tag="ps")
                    for ch in range(NCH):
                        for blk in range(GRP // 512):
                            nc.tensor.matmul(
                                ps[:, blk * 512:(blk + 1) * 512],
                                lhsT=ct2_t[ch][:, kt * 128:(kt + 1) * 128],
                                rhs=xg[ch][blk][:],
                                start=(ch == 0),
                                stop=(ch == NCH - 1),
                            )
                    if True:
                        col = kt * NG + g
                        # ~1 direct-psum reduce per group, rest evac+fold
                        if kt == (g % KT):
                            nc.vector.tensor_reduce(
                                out=bmax_t[:, col:col + 1],
                                in_=ps[:],
                                axis=mybir.AxisListType.X,
                                op=mybir.AluOpType.max,
                            )
                        else:
                            ev = spool.tile([128, GRP], mybir.dt.float16,
                                            tag="ev")
                            nc.scalar.copy(ev[:], ps[:])
                            f1 = spool.tile([128, GRP // 2], mybir.dt.float16,
                                            tag="f1")
                            nc.vector.tensor_tensor(
                                out=f1[:], in0=ev[:, 0:GRP // 2],
                                in1=ev[:, GRP // 2:GRP],
                                op=mybir.AluOpType.max)
                            f2 = spool.tile([128, GRP // 4], mybir.dt.float16,
                                            tag="f2")
                            nc.vector.tensor_tensor(
                                out=f2[:], in0=f1[:, 0:GRP // 4],
                                in1=f1[:, GRP // 4:GRP // 2],
                                op=mybir.AluOpType.max)
                            f3 = spool.tile([128, GRP // 8], mybir.dt.float16,
                                            tag="f3")
                            nc.vector.tensor_tensor(
                                out=f3[:], in0=f2[:, 0:GRP // 8],
                                in1=f2[:, GRP // 8:GRP // 4],
                                op=mybir.AluOpType.max)
                            nc.vector.tensor_reduce(
                                out=bmax_t[:, col:col + 1],
                                in_=f3[:],
                                axis=mybir.AxisListType.X,
                                op=mybir.AluOpType.max,
                            )

            nc.sync.dma_start(bmax_d[:, :], bmax_t[:])

    nc.compile()
    return nc


def host_prep(x, cluster_centers):
    """Sort points by |x|^2; build per-core device inputs."""
    x0 = np.ascontiguousarray(x[0], dtype=np.float32)        # (N, F)
    C = np.ascontiguousarray(cluster_centers, dtype=np.float32)
    x2 = np.einsum('nf,nf->n', x0.astype(np.float64),
                   x0.astype(np.float64))
    order = np.argsort(x2, kind="stable").astype(np.int64)
    xs_all = x0[order]                                        # sorted points
    x2s = x2[order]
    ct2_np = np.ascontiguousarray(
        (2.0 * C).T.astype(BF16)).reshape(NCH, 128, K)
    in_maps = []
    for c in range(NCORES):
        xs = xs_all[c * SH:(c + 1) * SH]
        xt_np = np.ascontiguousarray(xs.T.astype(BF16)).reshape(NCH, 128, SH)
        in_maps.append({"xt": xt_np, "ct2": ct2_np})
    return in_maps, x0, C, order, xs_all, x2s


def host_combine(bmax_cores, x0, C, order, xs_all, x2s):
    """Exact argmin recovery from per-group maxima of 2*dot (sorted points)."""
    x64s = xs_all.astype(np.float64)
    C64 = C.astype(np.float64)
    x2s_32 = x2s.astype(np.float32)

    # bmax_cores[c]: [128, KT*NG] -> cluster k = kt*128 + p, col = kt*NG + g
    bm = np.empty((K, NGRP), dtype=np.float32)
    for c in range(NCORES):
        a = np.asarray(bmax_cores[c]).reshape(128, KT, NG)
        bm[:, c * NG:(c + 1) * NG] = a.transpose(1, 0, 2).reshape(K, NG)

    gb = np.arange(NGRP) * GRP
    x2min = x2s[gb].astype(np.float32)            # sorted -> min is first
    x2max = x2s[gb + GRP - 1].astype(np.float32)

    ub = bm - x2min[None, :]                      # >= true group smax
    lb = bm - x2max[None, :]                      # <= true group smax
    win_lb = lb.max(axis=1)
    flags = ub >= (win_lb[:, None] - THETA)       # (K, NGRP)

    pair_clusters = [[] for _ in range(NGRP)]
    ks_idx, ps_idx = np.nonzero(flags)
    for kk, p in zip(ks_idx, ps_idx):
        pair_clusters[p].append(kk)

    best_val = np.full(K, np.inf)
    best_idx = np.zeros(K, dtype=np.int64)        # original indices
    for p, ks in enumerate(pair_clusters):
        if not ks:
            continue
        base = p * GRP
        pts = xs_all[base:base + GRP]
        d32 = x2s_32[base:base + GRP, None] - 2.0 * (pts @ C[ks].T)
        m = min(TOPM, GRP - 1)
        part = np.argpartition(d32, m, axis=0)[:m]
        for j, kk in enumerate(ks):
            srt = base + part[:, j]
            dv = x2s[srt] - 2.0 * (x64s[srt] @ C64[kk])
            ids = order[srt]                      # original indices
            o = np.lexsort((ids, dv))[0]
            if (dv[o] < best_val[kk]) or (dv[o] == best_val[kk]
                                          and ids[o] < best_idx[kk]):
                best_val[kk] = dv[o]
                best_idx[kk] = ids[o]

    return x0[best_idx][None].astype(np.float32)


_NC_CACHE = {}


def kernel(x, cluster_centers):
    from concourse.bass_utils import run_bass_kernel_spmd

    if "nc" not in _NC_CACHE:
        _NC_CACHE["nc"] = build_nc()
    nc = _NC_CACHE["nc"]

    in_maps, x0, C, order, xs_all, x2s = host_prep(x, cluster_centers)
    res = run_bass_kernel_spmd(nc, in_maps, list(range(NCORES)))
    bmax_cores = [res.results[c]["bmax"] for c in range(NCORES)]
    return host_combine(bmax_cores, x0, C, order, xs_all, x2s)
